# revision 3
# baseline (speedup 1.0000x reference)
"""DeepPoly ReLU abstract-transformer kernel for 8 TRN2 NeuronCores.

Reference semantics (elementwise over N = 16,777,216):
    x_out     = relu(x)
    neg  = upper <= 0          -> bounds (0, 0)
    pos  = lower >= 0          -> bounds (upper, upper)
    crossing   (else)          -> (lower, upper^2 / (upper - lower))

The problem is pure-elementwise and memory-bound; the harness tolerance is
rel_err < 2e-2, which leaves room to move all six DRAM tensors in bf16
(worst-case rounding ~4e-3) and halve HBM traffic: 12 B/elem instead of 24.
The host converts f32 -> bf16 before upload and bf16 -> f32 after download;
all device arithmetic that matters (denominator, square, reciprocal) runs
in f32 on-chip.

Branch-free device formulation per tile (inputs x, l, u in bf16):
    x_out = relu(x)                           # ACT, in place
    lmin  = min(l, 0)                         # GPSIMD ts
    d     = (u max 0) - lmin  -> f32          # GPSIMD stt: u+ + relu(-l)
    r     = 1/d                               # DVE reciprocal_approx_fast
    sq    = (u max 0) * u     -> f32          # DVE stt: relu(u)^2
    uo    = sq * r            -> bf16         # DVE
      neg: 0*(1/-l)=0; pos: u^2/u=u; crossing: u^2/(u-l)
    pp    = (l >= 0) u8                       # DVE ts
    le    = (u <= 0) u8                       # GPSIMD ts
    lower_out (in place on l):
      where(le) <- 0                          # DVE copy_predicated
      where(pp) <- u                          # DVE copy_predicated (exact)

Sharding: split N across the 8 cores; each core sees a [128, 16384] bf16
view of its 2,097,152-element slice. No communication.
"""

import numpy as np
import ml_dtypes

import concourse.bacc as bacc
import concourse.mybir as mybir
import concourse.tile as tile
from concourse import bass_utils

N_CORES = 8
N_TOTAL = 16777216
P = 128
NCOLS = N_TOTAL // N_CORES // P  # 16384
TILE_F = 4096
BUFS = 2
LAYOUT = "flat"
IN_DMA = "sync"
OUT_DMA = "scalar"

_BF16 = mybir.dt.bfloat16
_F32 = mybir.dt.float32
_U8 = mybir.dt.uint8
_RELU = mybir.ActivationFunctionType.Relu
_OP = mybir.AluOpType


def build_nc(
    ncols: int = NCOLS,
    tile_f: int = TILE_F,
    bufs: int = BUFS,
    reps: int = 1,
    layout: str = LAYOUT,
    in_dma: str = IN_DMA,
    out_dma: str = OUT_DMA,
):
    """reps > 1 repeats the whole pipeline in one NEFF (benchmarking only:
    lets wall-clock deltas cancel the per-launch dispatch overhead).
    layout: "flat" = [P, ncols] DRAM tensors, tiles are column slices;
    "contig" = [ntiles, P, tile_f] so each tile is one dense DRAM block.
    in_dma/out_dma: engine issuing the input/output DMAs ("sync"/"scalar"
    are the two HWDGE rings; "gpsimd" is SWDGE)."""
    assert ncols % tile_f == 0
    ntiles = ncols // tile_f
    nc = bacc.Bacc(
        "TRN2", target_bir_lowering=False, debug=False, num_devices=N_CORES
    )
    shape = [P, ncols] if layout == "flat" else [ntiles, P, tile_f]
    x = nc.dram_tensor("x", shape, _BF16, kind="ExternalInput").ap()
    lo = nc.dram_tensor("lower", shape, _BF16, kind="ExternalInput").ap()
    up = nc.dram_tensor("upper", shape, _BF16, kind="ExternalInput").ap()
    xo = nc.dram_tensor("x_out", shape, _BF16, kind="ExternalOutput").ap()
    loo = nc.dram_tensor("lower_out", shape, _BF16, kind="ExternalOutput").ap()
    upo = nc.dram_tensor("upper_out", shape, _BF16, kind="ExternalOutput").ap()

    def tslice(t, i):
        if layout == "flat":
            return t[:, i * tile_f : (i + 1) * tile_f]
        return t[i]

    ieng = lambda: getattr(nc, in_dma)
    oeng = lambda: getattr(nc, out_dma)

    with tile.TileContext(nc) as tc:
        with (
            tc.tile_pool(name="const", bufs=1) as cpool,
            tc.tile_pool(name="io", bufs=bufs) as pool,
        ):
            zt = cpool.tile([P, tile_f], _BF16, tag="zero")
            nc.gpsimd.memset(zt[:], 0.0)

            def one_iter(i):
                xt = pool.tile([P, tile_f], _BF16, tag="x")
                lt = pool.tile([P, tile_f], _BF16, tag="l")
                ut = pool.tile([P, tile_f], _BF16, tag="u")
                ieng().dma_start(out=xt[:], in_=tslice(x, i))
                ieng().dma_start(out=lt[:], in_=tslice(lo, i))
                ieng().dma_start(out=ut[:], in_=tslice(up, i))

                nc.scalar.activation(xt[:], xt[:], _RELU)  # x_out, in place

                # masks first (exact comparisons on the bf16 inputs)
                ppt = pool.tile([P, tile_f], _U8, tag="pp")
                nc.vector.tensor_scalar(
                    out=ppt[:], in0=lt[:], scalar1=0.0, scalar2=None,
                    op0=_OP.is_ge,
                )
                let = pool.tile([P, tile_f], _U8, tag="le")
                nc.gpsimd.tensor_scalar(
                    out=let[:], in0=ut[:], scalar1=0.0, scalar2=None,
                    op0=_OP.is_le,
                )

                # d = relu(u) + relu(-l) = (u max 0) - min(l, 0)   (f32)
                lmt = pool.tile([P, tile_f], _BF16, tag="lm")
                nc.gpsimd.tensor_scalar(
                    out=lmt[:], in0=lt[:], scalar1=0.0, scalar2=None,
                    op0=_OP.min,
                )
                dt = pool.tile([P, tile_f], _F32, tag="d")
                nc.vector.scalar_tensor_tensor(
                    out=dt[:], in0=ut[:], scalar=0.0, in1=lmt[:],
                    op0=_OP.max, op1=_OP.subtract,
                )
                nc.vector.reciprocal_approx_fast(out=dt[:], in_=dt[:])

                # sq = relu(u)^2 = (u max 0) * u   (f32)
                sqt = pool.tile([P, tile_f], _F32, tag="sq")
                nc.vector.scalar_tensor_tensor(
                    out=sqt[:], in0=ut[:], scalar=0.0, in1=ut[:],
                    op0=_OP.max, op1=_OP.mult,
                )
                uot = pool.tile([P, tile_f], _BF16, tag="uo")
                nc.vector.tensor_mul(out=uot[:], in0=sqt[:], in1=dt[:])

                nc.vector.copy_predicated(out=lt[:], mask=let[:], data=zt[:])
                nc.vector.copy_predicated(out=lt[:], mask=ppt[:], data=ut[:])

                oeng().dma_start(out=tslice(xo, i), in_=xt[:])
                oeng().dma_start(out=tslice(loo, i), in_=lt[:])
                oeng().dma_start(out=tslice(upo, i), in_=uot[:])

            def body():
                for i in range(ntiles):
                    one_iter(i)

            if reps == 1:
                body()
            else:
                # benchmarking only: hardware loop keeps the body IRAM-resident
                with tc.For_i(0, reps, 1):
                    body()
    nc.compile()
    return nc


def _to_bf16_shards(inputs: dict, layout: str = LAYOUT, tile_f: int = TILE_F):
    """f32 host arrays -> per-core bf16 arrays in the device DRAM layout."""
    if layout == "flat":
        core_shape = (P, NCOLS)
    else:
        core_shape = (NCOLS // tile_f, P, tile_f)
    arrs = {}
    for k in ("x", "lower", "upper"):
        a = np.asarray(inputs[k], dtype=np.float32)
        a = np.ascontiguousarray(a).astype(ml_dtypes.bfloat16)
        arrs[k] = a.reshape(N_CORES, *core_shape)
    return arrs


def run(inputs: dict, trace: bool = False):
    """Shard, execute on 8 cores, gather. Returns (outputs_tuple, results_obj)."""
    arrs = _to_bf16_shards(inputs)
    in_maps = [
        {k: arrs[k][c] for k in ("x", "lower", "upper")} for c in range(N_CORES)
    ]
    nc = build_nc()
    res = bass_utils.run_bass_kernel_spmd(
        nc, in_maps, core_ids=list(range(N_CORES)), trace=trace
    )
    outs = []
    for name in ("x_out", "lower_out", "upper_out"):
        full = np.stack([res.results[c][name] for c in range(N_CORES)])
        outs.append(full.reshape(1, N_TOTAL).astype(np.float32))
    return tuple(outs), res


def kernel(**inputs):
    outs, _ = run(inputs, trace=False)
    return outs


# revision 5
# speedup vs baseline: 5.8317x; 5.8317x over previous
"""DeepPoly ReLU abstract-transformer kernel for 8 TRN2 NeuronCores.

Reference semantics (elementwise over N = 16,777,216):
    x_out     = relu(x)
    neg  = upper <= 0          -> bounds (0, 0)
    pos  = lower >= 0          -> bounds (upper, upper)
    crossing   (else)          -> (lower, upper^2 / (upper - lower))

The problem is pure-elementwise and memory-bound; the harness tolerance is
rel_err < 2e-2, which leaves room to move all six DRAM tensors in bf16
(worst-case rounding ~4e-3) and halve HBM traffic: 12 B/elem instead of 24.
The host converts f32 -> bf16 before upload and bf16 -> f32 after download;
all device arithmetic that matters (denominator, square, reciprocal) runs
in f32 on-chip.

Branch-free device formulation per tile (inputs x, l, u in bf16):
    x_out = relu(x)                           # ACT, in place
    lmin  = min(l, 0)                         # GPSIMD ts
    d     = (u max 0) - lmin  -> f32          # GPSIMD stt: u+ + relu(-l)
    r     = 1/d                               # DVE reciprocal_approx_fast
    sq    = (u max 0) * u     -> f32          # DVE stt: relu(u)^2
    uo    = sq * r            -> bf16         # DVE
      neg: 0*(1/-l)=0; pos: u^2/u=u; crossing: u^2/(u-l)
    pp    = (l >= 0) u8                       # DVE ts
    le    = (u <= 0) u8                       # GPSIMD ts
    lower_out (in place on l):
      where(le) <- 0                          # DVE copy_predicated
      where(pp) <- u                          # DVE copy_predicated (exact)

Sharding: split N across the 8 cores; each core sees a [128, 16384] bf16
view of its 2,097,152-element slice. No communication.
"""

import numpy as np
import ml_dtypes

import concourse.bacc as bacc
import concourse.mybir as mybir
import concourse.tile as tile
from concourse import bass_utils

N_CORES = 8
N_TOTAL = 16777216
P = 128
NCOLS = N_TOTAL // N_CORES // P  # 16384
TILE_F = 4096
BUFS = 2
LAYOUT = "flat"
IN_DMA = "sync"
OUT_DMA = "scalar"
COMPUTE = "v2"  # "v2" = gpsimd masks; "dve" = all elementwise on DVE

_BF16 = mybir.dt.bfloat16
_F32 = mybir.dt.float32
_U8 = mybir.dt.uint8
_RELU = mybir.ActivationFunctionType.Relu
_OP = mybir.AluOpType


def build_nc(
    ncols: int = NCOLS,
    tile_f: int = TILE_F,
    bufs: int = BUFS,
    reps: int = 1,
    layout: str = LAYOUT,
    in_dma: str = IN_DMA,
    out_dma: str = OUT_DMA,
    compute: str = COMPUTE,
):
    """reps > 1 repeats the whole pipeline in one NEFF (benchmarking only:
    lets wall-clock deltas cancel the per-launch dispatch overhead).
    layout: "flat" = [P, ncols] DRAM tensors, tiles are column slices;
    "contig" = [ntiles, P, tile_f] so each tile is one dense DRAM block.
    in_dma/out_dma: engine issuing the input/output DMAs ("sync"/"scalar"
    are the two HWDGE rings; "gpsimd" is SWDGE)."""
    assert ncols % tile_f == 0
    ntiles = ncols // tile_f
    nc = bacc.Bacc(
        "TRN2", target_bir_lowering=False, debug=False, num_devices=N_CORES
    )
    shape = [P, ncols] if layout == "flat" else [ntiles, P, tile_f]
    x = nc.dram_tensor("x", shape, _BF16, kind="ExternalInput").ap()
    lo = nc.dram_tensor("lower", shape, _BF16, kind="ExternalInput").ap()
    up = nc.dram_tensor("upper", shape, _BF16, kind="ExternalInput").ap()
    xo = nc.dram_tensor("x_out", shape, _BF16, kind="ExternalOutput").ap()
    loo = nc.dram_tensor("lower_out", shape, _BF16, kind="ExternalOutput").ap()
    upo = nc.dram_tensor("upper_out", shape, _BF16, kind="ExternalOutput").ap()

    def tslice(t, i):
        if layout == "flat":
            return t[:, i * tile_f : (i + 1) * tile_f]
        return t[i]

    ieng = lambda: getattr(nc, in_dma)
    oeng = lambda: getattr(nc, out_dma)

    with tile.TileContext(nc) as tc:
        with (
            tc.tile_pool(name="const", bufs=1) as cpool,
            tc.tile_pool(name="io", bufs=bufs) as pool,
        ):
            zt = cpool.tile([P, tile_f], _BF16, tag="zero")
            nc.gpsimd.memset(zt[:], 0.0)

            def one_iter(i):
                xt = pool.tile([P, tile_f], _BF16, tag="x")
                lt = pool.tile([P, tile_f], _BF16, tag="l")
                ut = pool.tile([P, tile_f], _BF16, tag="u")
                ieng().dma_start(out=xt[:], in_=tslice(x, i))
                ieng().dma_start(out=lt[:], in_=tslice(lo, i))
                ieng().dma_start(out=ut[:], in_=tslice(up, i))

                nc.scalar.activation(xt[:], xt[:], _RELU)  # x_out, in place

                # masks first (exact comparisons on the bf16 inputs)
                ppt = pool.tile([P, tile_f], _U8, tag="pp")
                nc.vector.tensor_scalar(
                    out=ppt[:], in0=lt[:], scalar1=0.0, scalar2=None,
                    op0=_OP.is_ge,
                )
                let = pool.tile([P, tile_f], _U8, tag="le")
                mask_eng = nc.vector if compute == "dve" else nc.gpsimd
                mask_eng.tensor_scalar(
                    out=let[:], in0=ut[:], scalar1=0.0, scalar2=None,
                    op0=_OP.is_le,
                )

                # d = relu(u) + relu(-l) = (u max 0) - min(l, 0)   (f32)
                lmt = pool.tile([P, tile_f], _BF16, tag="lm")
                mask_eng.tensor_scalar(
                    out=lmt[:], in0=lt[:], scalar1=0.0, scalar2=None,
                    op0=_OP.min,
                )
                dt = pool.tile([P, tile_f], _F32, tag="d")
                nc.vector.scalar_tensor_tensor(
                    out=dt[:], in0=ut[:], scalar=0.0, in1=lmt[:],
                    op0=_OP.max, op1=_OP.subtract,
                )
                nc.vector.reciprocal_approx_fast(out=dt[:], in_=dt[:])

                # sq = relu(u)^2 = (u max 0) * u   (f32)
                sqt = pool.tile([P, tile_f], _F32, tag="sq")
                nc.vector.scalar_tensor_tensor(
                    out=sqt[:], in0=ut[:], scalar=0.0, in1=ut[:],
                    op0=_OP.max, op1=_OP.mult,
                )
                uot = pool.tile([P, tile_f], _BF16, tag="uo")
                nc.vector.tensor_mul(out=uot[:], in0=sqt[:], in1=dt[:])

                nc.vector.copy_predicated(out=lt[:], mask=let[:], data=zt[:])
                nc.vector.copy_predicated(out=lt[:], mask=ppt[:], data=ut[:])

                oeng().dma_start(out=tslice(xo, i), in_=xt[:])
                oeng().dma_start(out=tslice(loo, i), in_=lt[:])
                oeng().dma_start(out=tslice(upo, i), in_=uot[:])

            def body():
                for i in range(ntiles):
                    one_iter(i)

            if reps == 1:
                body()
            else:
                # benchmarking only: hardware loop keeps the body IRAM-resident
                with tc.For_i(0, reps, 1):
                    body()
    nc.compile()
    return nc


def _to_bf16_shards(inputs: dict, layout: str = LAYOUT, tile_f: int = TILE_F):
    """f32 host arrays -> per-core bf16 arrays in the device DRAM layout."""
    if layout == "flat":
        core_shape = (P, NCOLS)
    else:
        core_shape = (NCOLS // tile_f, P, tile_f)
    arrs = {}
    for k in ("x", "lower", "upper"):
        a = np.asarray(inputs[k], dtype=np.float32)
        a = np.ascontiguousarray(a).astype(ml_dtypes.bfloat16)
        arrs[k] = a.reshape(N_CORES, *core_shape)
    return arrs


def run(inputs: dict, trace: bool = False):
    """Shard, execute on 8 cores, gather. Returns (outputs_tuple, results_obj)."""
    arrs = _to_bf16_shards(inputs)
    in_maps = [
        {k: arrs[k][c] for k in ("x", "lower", "upper")} for c in range(N_CORES)
    ]
    nc = build_nc()
    res = bass_utils.run_bass_kernel_spmd(
        nc, in_maps, core_ids=list(range(N_CORES)), trace=trace
    )
    outs = []
    for name in ("x_out", "lower_out", "upper_out"):
        full = np.stack([res.results[c][name] for c in range(N_CORES)])
        outs.append(full.reshape(1, N_TOTAL).astype(np.float32))
    return tuple(outs), res


def kernel(**inputs):
    outs, _ = run(inputs, trace=False)
    return outs
